# revision 19
# baseline (speedup 1.0000x reference)
"""Self-contained Trainium2 kernel for nn_BanzhafModule (conv1 -> self-attention -> conv2).

Data-parallel over 8 NeuronCores: each core processes 4 of the 32 (b*a) batch
elements end-to-end; no collectives.

Algebraic folds (host-side, exact):
  * S = (hx Q)(hx K)^T = hx (Q K^T) hx^T  -> precompute M = Q K^T; the K
    projection disappears (scores contract qt' = M^T hx^T against ht directly).
  * o enters the output only through conv2:  p9o = W2^T o^T
    = (W2^T V^T) (hx^T E^T)  -> precompute W2V = V @ W2r; the V projection and
    the O matmul are replaced by A = hx^T E^T (fp8 DoubleRow) and a tiny
    [9 x L] contraction.

Precision: conv1 / qt' / scores / p9h in fp32r; exp probs (normalized by the
rowsum before the transpose, as a per-partition scale in M-layout) + token-major
h in fp8e4; A copy + p9oA in bf16; taps in bf16.  b1 rides in the conv as a
10th "ones" tap row.  PSUM accumulation groups never share a 2KB bank.
"""

import numpy as np

E = 4          # batch elements per core
NCORES = 8
IMG = 32       # t = v = 32
L = IMG * IMG  # 1024 tokens
P = 512        # planes

_TAPS = [(dy, dx) for dy in range(3) for dx in range(3)]

_built = {}


def _build_nc():
    import os
    STAGE = int(os.environ.get("KSTAGE", "99"))
    import concourse.mybir as mybir
    from concourse import bacc
    from concourse.tile import TileContext
    from concourse.masks import make_identity

    f32, f32r, bf16 = mybir.dt.float32, mybir.dt.float32r, mybir.dt.bfloat16
    fp8 = mybir.dt.float8e4
    AF = mybir.ActivationFunctionType
    ALU = mybir.AluOpType
    AX = mybir.AxisListType
    DR = mybir.MatmulPerfMode.DoubleRow

    nc = bacc.Bacc("TRN2", target_bir_lowering=False, debug=False, num_devices=NCORES)

    i_xcol = nc.dram_tensor("xcol", [E, 10, L], bf16, kind="ExternalInput")
    i_w1 = nc.dram_tensor("W1cb", [10, P], bf16, kind="ExternalInput")
    i_m = nc.dram_tensor("Mm", [128, 4, P], bf16, kind="ExternalInput")
    i_w2 = nc.dram_tensor("W2m", [128, 8, 9], bf16, kind="ExternalInput")
    i_b2 = nc.dram_tensor("b2v", [1, 1], f32, kind="ExternalInput")
    o_out = nc.dram_tensor("out", [E, L], f32, kind="ExternalOutput")

    ones_col_d = nc.inline_tensor(np.ones((128, 1), np.float32), name="ones_col")

    with TileContext(nc) as tc:
        with (
            tc.tile_pool(name="wts", bufs=1) as wts,
            tc.tile_pool(name="hp", bufs=2) as hp,
            tc.tile_pool(name="htp", bufs=2) as htp,
            tc.tile_pool(name="qp", bufs=2) as qp,
            tc.tile_pool(name="ep", bufs=2) as ep,
            tc.tile_pool(name="xp", bufs=2) as xp,
            tc.tile_pool(name="msc", bufs=2) as msc,
            tc.tile_pool(name="fin", bufs=1) as fin,
            tc.tile_pool(name="pmm", bufs=3, space="PSUM") as pmm,
            tc.tile_pool(name="ptp", bufs=2, space="PSUM") as ptp,
            tc.tile_pool(name="xm", bufs=2) as xm,
            tc.tile_pool(name="xn", bufs=2) as xn,
        ):
            # ---- weights / constants (persistent, DMA'd straight in) ----
            prefetch = {}
            xcf0 = xp.tile([10, L], bf16, tag="xcol", name="xcf0")
            nc.sync.dma_start(xcf0[:], i_xcol.ap()[0])
            prefetch[0] = xcf0

            w1c = wts.tile([10, P], bf16, tag="w1c")
            nc.gpsimd.dma_start(w1c[:], i_w1.ap())
            mm = wts.tile([128, 4, P], bf16, tag="mm", name="mm")
            nc.sync.dma_start(mm[:], i_m.ap())
            w2f = wts.tile([128, 8, 9], bf16, tag="w2f")
            nc.sync.dma_start(w2f[:], i_w2.ap())

            onc = wts.tile([128, 1], f32)
            nc.sync.dma_start(onc[:], ones_col_d.ap())
            oncb = wts.tile([128, 1], bf16)
            nc.vector.tensor_copy(oncb[:], onc[:])
            identb = wts.tile([128, 128], bf16)
            make_identity(nc, identb[:])

            b2t = wts.tile([1, 1], f32)
            nc.sync.dma_start(b2t[:], i_b2.ap())
            p9sh = fin.tile([9, E, L], bf16)
            nc.gpsimd.memset(p9sh[:], 0.0)

            state = {}

            def conv1_proj(e):
                """conv1 (both layouts) + qt' = M^T hx^T projection."""
                xc = prefetch.pop(e, None)
                if xc is None:
                    xc = xp.tile([10, L], bf16, tag="xcol")
                    nc.sync.dma_start(xc[:], i_xcol.ap()[e])
                # channel-major: ht[p, l] = relu(sum_j w1cb[j, p] xc10[j, l])
                ht = hp.tile([128, 8, L], bf16, tag="H")
                for ck in range(4):
                    ps = pmm.tile([128, 1024], f32, tag="pmm")
                    for lg in range(2):
                        nc.tensor.matmul(
                            ps[:, lg * 512:(lg + 1) * 512],
                            w1c[:, ck * 128:(ck + 1) * 128],
                            xc[:, lg * 512:(lg + 1) * 512],
                            start=True, stop=True,
                        )
                    if ck % 2 == 0:
                        nc.scalar.activation(ht[:, ck, :], ps[:], AF.Relu)
                    else:
                        nc.vector.tensor_scalar_max(ht[:, ck, :], ps[:], 0.0)
                # qt'[n, l] = sum_d M[d, n] ht[d, l]
                qt = qp.tile([128, 4, L], bf16, tag="qT")
                for nck in range(4):
                    ps = pmm.tile([128, 1024], f32, tag="pmm")
                    for lg in range(2):
                        for dk in range(4):
                            nc.tensor.matmul(
                                ps[:, lg * 512:(lg + 1) * 512],
                                mm[:, dk, nck * 128:(nck + 1) * 128],
                                ht[:, dk, lg * 512:(lg + 1) * 512],
                                start=(dk == 0), stop=(dk == 3),
                            )
                    if nck == 3:
                        # split the last copy across both engines: scores(lc=0)
                        # stalls on exactly this tile
                        nc.scalar.copy(qt[:, nck, 0:512], ps[:, 0:512])
                        nc.vector.tensor_copy(qt[:, nck, 512:1024], ps[:, 512:1024])
                    elif nck % 2 == 0:
                        nc.scalar.copy(qt[:, nck, :], ps[:])
                    else:
                        nc.vector.tensor_copy(qt[:, nck, :], ps[:])
                # token-major: hT8[l, lc, p] = relu(sum_j xc10[j, l] w1cb[j, p]) in fp8
                ht8 = htp.tile([128, 8, P], fp8, tag="HT8")
                for lcp in range(4):
                    ps = pmm.tile([128, 1024], f32, tag="pmm")
                    for half in range(2):
                        lc = lcp * 2 + half
                        nc.tensor.matmul(
                            ps[:, half * 512:(half + 1) * 512],
                            xc[:, lc * 128:(lc + 1) * 128],
                            w1c[:],
                            start=True, stop=True,
                        )
                    dst = ht8[:, lcp * 2:lcp * 2 + 2, :].rearrange(
                        "p c w -> p (c w)"
                    )
                    if lcp % 2 == 0:
                        nc.scalar.activation(dst, ps[:], AF.Relu)
                    else:
                        nc.vector.tensor_scalar_max(dst, ps[:], 0.0)
                state[e] = (ht, ht8, qt)

            def finalsum(e):
                """Sum the 9 scattered tap rows on PE, add b2, DMA out."""
                acc1 = msc.tile([1, L], f32, tag="acc1")
                psf = pmm.tile([1, 1024], f32, tag="pmm", name="psf")
                for lg in range(2):
                    sl = slice(lg * 512, (lg + 1) * 512)
                    nc.tensor.matmul(
                        psf[:, sl], oncb[0:9, 0:1], p9sh[0:9, e, sl],
                        start=True, stop=True,
                    )
                nc.scalar.activation(
                    acc1[:], psf[:], AF.Identity, bias=b2t[0:1, 0:1]
                )
                nc.sync.dma_start(o_out.ap()[e:e + 1, :], acc1[0:1, :])

            def attention(e):
                ht, ht8, qt = state[e]
                if STAGE < 2:
                    if e + 1 < E:
                        conv1_proj(e + 1)
                    return
                # ---- scores in M-layout; exp with fused -max bias and rowsum;
                #      normalize by 1/rowsum (per-partition scale) and
                #      PE-transpose each 128x128 prob tile into T-layout ----
                nmcol = msc.tile([128, 8], f32, tag="nmcol")
                rscol = msc.tile([128, 8], f32, tag="rscol")
                rcol = msc.tile([128, 8], f32, tag="rcol")
                et = ep.tile([128, 8, L], fp8, tag="eT")
                for lc in range(8):
                    ps = pmm.tile([128, 1024], f32, tag="pmm")
                    for mg in range(2):
                        for ncx in range(4):
                            nc.tensor.matmul(
                                ps[:, mg * 512:(mg + 1) * 512],
                                qt[:, ncx, lc * 128:(lc + 1) * 128],
                                ht[:, ncx, mg * 512:(mg + 1) * 512],
                                start=(ncx == 0), stop=(ncx == 3),
                            )
                    nc.vector.tensor_reduce(
                        nmcol[:, lc:lc + 1], ps[:], axis=AX.X, op=ALU.max, negate=True
                    )
                    expm = xm.tile([128, 1024], bf16, tag="expM")
                    nc.scalar.activation(
                        expm[:], ps[:], AF.Exp,
                        bias=nmcol[:, lc:lc + 1],
                        accum_out=rscol[:, lc:lc + 1],
                    )
                    nc.vector.reciprocal(rcol[:, lc:lc + 1], rscol[:, lc:lc + 1])
                    expn = xn.tile([128, 1024], bf16, tag="expN")
                    if lc % 2 == 0:
                        nc.scalar.activation(
                            expn[:], expm[:], AF.Identity,
                            scale=rcol[:, lc:lc + 1],
                        )
                    else:
                        nc.vector.tensor_scalar(
                            expn[:], expm[:], rcol[:, lc:lc + 1], None, ALU.mult
                        )
                    ptr = ptp.tile([128, 1024], bf16, tag="ptr")
                    for mc in range(8):
                        nc.tensor.transpose(
                            ptr[:, mc * 128:(mc + 1) * 128],
                            expn[:, mc * 128:(mc + 1) * 128],
                            identb[:],
                        )
                    for mc in range(0, 8, 2):
                        dst = et[:, mc:mc + 2, lc * 128:(lc + 1) * 128]
                        srcp = ptr[:, mc * 128:(mc + 2) * 128].rearrange(
                            "p (c w) -> p c w", c=2
                        )
                        if mc % 4 == 0:
                            nc.scalar.copy(dst, srcp)
                        else:
                            nc.vector.tensor_copy(dst, srcp)

                # final tap-sum of the previous element: the gpsimd/sync scatter
                # for it finished during the scores block, so PE never stalls
                if e > 0 and STAGE >= 8:
                    finalsum(e - 1)

                if STAGE < 4:
                    if e + 1 < E:
                        conv1_proj(e + 1)
                    return
                # ---- A[d, q] = sum_k hT8[k, d] Enorm^T[k, q]  (fp8 DoubleRow) ----
                # PSUM "start" zeroes a whole 2KB bank, so the four concurrent
                # 256-col accumulation groups must each own a distinct bank:
                # qg 0/2 -> psx banks 0/1, qg 1/3 -> psy banks 0/1.
                for dc in range(4):
                    psx = pmm.tile([128, 1024], f32, tag="pmm", name="psxA")
                    psy = pmm.tile([128, 1024], f32, tag="pmm", name="psyA")
                    for kc in range(0, 8, 2):
                        for qg in range(4):
                            ps = psx if qg % 2 == 0 else psy
                            col = (qg // 2) * 512
                            nc.tensor.matmul(
                                ps[:, col:col + 256],
                                ht8[:, kc:kc + 2, dc * 128:(dc + 1) * 128],
                                et[:, kc:kc + 2, qg * 256:(qg + 1) * 256],
                                start=(kc == 0), stop=(kc == 6),
                                perf_mode=DR,
                            )
                    for src, qoff in ((psx, 0), (psy, 256)):
                        s = src[:, :].rearrange("p (g w) -> p g w", g=2)[:, :, 0:256]
                        d = ht[:, 4 + dc, :].rearrange(
                            "p (g w) -> p g w", g=2
                        )[:, :, qoff:qoff + 256]
                        if qoff == 0:
                            nc.scalar.copy(d, s)
                        else:
                            nc.vector.tensor_copy(d, s)

                # next elem's prologue fills PE while conv2's DVE/DMA tail runs
                if e + 1 < E:
                    conv1_proj(e + 1)
                if STAGE < 6:
                    return
                # ---- conv2 taps: one 8-step chain over [H | A] chunks ----
                p9e = msc.tile([9, L], bf16, tag="p9e")
                p9ho = pmm.tile([9, 1024], f32, tag="pmm", name="p9ho")
                for lg in range(2):
                    sl = slice(lg * 512, (lg + 1) * 512)
                    for ck in range(8):
                        nc.tensor.matmul(
                            p9ho[:, sl], w2f[:, ck, :], ht[:, ck, sl],
                            start=(ck == 0), stop=(ck == 7),
                        )
                nc.scalar.copy(p9e[:], p9ho[:])
                if STAGE < 7:
                    return
                # scatter each tap row into its shifted, clipped window, split
                # across the gpsimd and sync DMA queues (byte-addressed, so the
                # unaligned partition bases are fine)
                for j, (dy, dx) in enumerate(_TAPS):
                    r0, r1 = max(0, 1 - dy), min(IMG, IMG + 1 - dy)
                    c0, c1 = max(0, 1 - dx), min(IMG, IMG + 1 - dx)
                    srcw = p9e[j:j + 1, :].rearrange("o (r w) -> o r w", w=IMG)[
                        :, r0 + dy - 1:r1 + dy - 1, c0 + dx - 1:c1 + dx - 1
                    ]
                    dstw = p9sh[j:j + 1, e, :].rearrange("o (r w) -> o r w", w=IMG)[
                        :, r0:r1, c0:c1
                    ]
                    if j % 3 == 0:
                        nc.gpsimd.dma_start(dstw, srcw)
                    else:
                        nc.sync.dma_start(dstw, srcw)

            conv1_proj(0)
            for e in range(E):
                attention(e)
            if STAGE >= 8:
                finalsum(E - 1)

    nc.compile()
    return nc


def _host_prep(x, W1, b1, Q, K, V, W2, b2):
    B = x.shape[0] * x.shape[1]
    xf = np.ascontiguousarray(x, np.float32).reshape(B, IMG, IMG)
    xpad = np.zeros((B, IMG + 2, IMG + 2), np.float32)
    xpad[:, 1:-1, 1:-1] = xf
    xcol = np.empty((B, 10, L), np.float32)
    for j, (dy, dx) in enumerate(_TAPS):
        xcol[:, j] = xpad[:, dy:dy + IMG, dx:dx + IMG].reshape(B, L)
    xcol[:, 9] = 1.0
    import ml_dtypes
    xcol = xcol.astype(ml_dtypes.bfloat16)
    w1r = np.asarray(W1, np.float32).reshape(P, 9)
    w1cb = np.ascontiguousarray(
        np.concatenate([w1r.T, np.asarray(b1, np.float32)[None, :]], axis=0)
    ).astype(ml_dtypes.bfloat16)  # [10, P]
    M = np.asarray(Q, np.float32) @ np.asarray(K, np.float32).T
    mm = np.ascontiguousarray(M.reshape(4, 128, P).transpose(1, 0, 2)).astype(
        ml_dtypes.bfloat16
    )
    w2r = np.asarray(W2, np.float32).reshape(P, 9)
    w2h = w2r.reshape(4, 128, 9).transpose(1, 0, 2)       # [128, 4, 9]
    w2vt_f = np.asarray(V, np.float32) @ w2r              # [512 d, 9 tap]
    w2v = w2vt_f.reshape(4, 128, 9).transpose(1, 0, 2)    # [128, 4, 9]
    w2m = np.ascontiguousarray(
        np.concatenate([w2h, w2v], axis=1)
    ).astype(ml_dtypes.bfloat16)                          # [128, 8, 9]
    b2v = np.asarray(b2, np.float32).reshape(1, 1)
    in_maps = []
    for c in range(NCORES):
        in_maps.append({
            "xcol": np.ascontiguousarray(xcol[E * c:E * (c + 1)]),
            "W1cb": w1cb, "Mm": mm, "W2m": w2m, "b2v": b2v,
        })
    return in_maps


def kernel(x, W1, b1, Q, K, V, W2, b2):
    from concourse.bass_utils import run_bass_kernel_spmd

    in_maps = _host_prep(x, W1, b1, Q, K, V, W2, b2)
    if "nc" not in _built:
        _built["nc"] = _build_nc()
    nc = _built["nc"]
    res = run_bass_kernel_spmd(nc, in_maps, core_ids=list(range(NCORES)))
    full = np.concatenate([res.results[c]["out"] for c in range(NCORES)], axis=0)
    return np.ascontiguousarray(
        full.reshape(x.shape[0], x.shape[1], IMG, IMG).astype(np.float32)
    )


# revision 20
# speedup vs baseline: 1.0176x; 1.0176x over previous
"""Self-contained Trainium2 kernel for nn_BanzhafModule (conv1 -> self-attention -> conv2).

Data-parallel over 8 NeuronCores: each core processes 4 of the 32 (b*a) batch
elements end-to-end; no collectives.

Algebraic folds (host-side, exact):
  * S = (hx Q)(hx K)^T = hx (Q K^T) hx^T  -> precompute M = Q K^T; the K
    projection disappears (scores contract qt' = M^T hx^T against ht directly).
  * o enters the output only through conv2:  p9o = W2^T o^T
    = (W2^T V^T) (hx^T E^T)  -> precompute W2V = V @ W2r; the V projection and
    the O matmul are replaced by A = hx^T E^T (fp8 DoubleRow) and a tiny
    [9 x L] contraction.

Precision: conv1 / qt' / scores / p9h in fp32r; exp probs (normalized by the
rowsum before the transpose, as a per-partition scale in M-layout) + token-major
h in fp8e4; A copy + p9oA in bf16; taps in bf16.  b1 rides in the conv as a
10th "ones" tap row.  PSUM accumulation groups never share a 2KB bank.
"""

import numpy as np

E = 4          # batch elements per core
NCORES = 8
IMG = 32       # t = v = 32
L = IMG * IMG  # 1024 tokens
P = 512        # planes

_TAPS = [(dy, dx) for dy in range(3) for dx in range(3)]

_built = {}


def _build_nc():
    import os
    STAGE = int(os.environ.get("KSTAGE", "99"))
    import concourse.mybir as mybir
    from concourse import bacc
    from concourse.tile import TileContext
    from concourse.masks import make_identity

    f32, f32r, bf16 = mybir.dt.float32, mybir.dt.float32r, mybir.dt.bfloat16
    fp8 = mybir.dt.float8e4
    AF = mybir.ActivationFunctionType
    ALU = mybir.AluOpType
    AX = mybir.AxisListType
    DR = mybir.MatmulPerfMode.DoubleRow

    nc = bacc.Bacc("TRN2", target_bir_lowering=False, debug=False, num_devices=NCORES)

    i_xcol = nc.dram_tensor("xcol", [E, 10, L], bf16, kind="ExternalInput")
    i_w1 = nc.dram_tensor("W1cb", [10, P], bf16, kind="ExternalInput")
    i_m = nc.dram_tensor("Mm", [128, 4, P], bf16, kind="ExternalInput")
    i_w2 = nc.dram_tensor("W2m", [128, 8, 9], bf16, kind="ExternalInput")
    i_b2 = nc.dram_tensor("b2v", [1, 1], f32, kind="ExternalInput")
    o_out = nc.dram_tensor("out", [E, L], f32, kind="ExternalOutput")

    ones_col_d = nc.inline_tensor(np.ones((128, 1), np.float32), name="ones_col")

    with TileContext(nc) as tc:
        with (
            tc.tile_pool(name="wts", bufs=1) as wts,
            tc.tile_pool(name="hp", bufs=2) as hp,
            tc.tile_pool(name="htp", bufs=2) as htp,
            tc.tile_pool(name="qp", bufs=2) as qp,
            tc.tile_pool(name="ep", bufs=2) as ep,
            tc.tile_pool(name="xp", bufs=2) as xp,
            tc.tile_pool(name="msc", bufs=2) as msc,
            tc.tile_pool(name="fin", bufs=1) as fin,
            tc.tile_pool(name="pmm", bufs=3, space="PSUM") as pmm,
            tc.tile_pool(name="ptp", bufs=2, space="PSUM") as ptp,
            tc.tile_pool(name="xm", bufs=2) as xm,
            tc.tile_pool(name="xn", bufs=2) as xn,
        ):
            # ---- weights / constants (persistent, DMA'd straight in) ----
            prefetch = {}
            xcf0 = xp.tile([10, L], bf16, tag="xcol", name="xcf0")
            nc.sync.dma_start(xcf0[:], i_xcol.ap()[0])
            prefetch[0] = xcf0

            w1c = wts.tile([10, P], bf16, tag="w1c")
            nc.gpsimd.dma_start(w1c[:], i_w1.ap())
            mm = wts.tile([128, 4, P], bf16, tag="mm", name="mm")
            nc.sync.dma_start(mm[:], i_m.ap())
            w2f = wts.tile([128, 8, 9], bf16, tag="w2f")
            nc.sync.dma_start(w2f[:], i_w2.ap())

            onc = wts.tile([128, 1], f32)
            nc.sync.dma_start(onc[:], ones_col_d.ap())
            oncb = wts.tile([128, 1], bf16)
            nc.vector.tensor_copy(oncb[:], onc[:])
            identb = wts.tile([128, 128], bf16)
            make_identity(nc, identb[:])

            b2t = wts.tile([1, 1], f32)
            nc.sync.dma_start(b2t[:], i_b2.ap())
            p9sh = fin.tile([9, E, L], bf16)
            nc.gpsimd.memset(p9sh[:], 0.0)

            state = {}

            def conv1_proj(e):
                """conv1 (both layouts) + qt' = M^T hx^T projection."""
                xc = prefetch.pop(e, None)
                if xc is None:
                    xc = xp.tile([10, L], bf16, tag="xcol")
                    nc.sync.dma_start(xc[:], i_xcol.ap()[e])
                # channel-major: ht[p, l] = relu(sum_j w1cb[j, p] xc10[j, l])
                ht = hp.tile([128, 8, L], bf16, tag="H")
                for ck in range(4):
                    ps = pmm.tile([128, 1024], f32, tag="pmm")
                    for lg in range(2):
                        nc.tensor.matmul(
                            ps[:, lg * 512:(lg + 1) * 512],
                            w1c[:, ck * 128:(ck + 1) * 128],
                            xc[:, lg * 512:(lg + 1) * 512],
                            start=True, stop=True,
                        )
                    if ck % 2 == 0:
                        nc.scalar.activation(ht[:, ck, :], ps[:], AF.Relu)
                    else:
                        nc.vector.tensor_scalar_max(ht[:, ck, :], ps[:], 0.0)
                # token-major: hT8[l, lc, p] = relu(sum_j xc10[j, l] w1cb[j, p]) in fp8
                ht8 = htp.tile([128, 8, P], fp8, tag="HT8")
                for lcp in range(4):
                    ps = pmm.tile([128, 1024], f32, tag="pmm")
                    for half in range(2):
                        lc = lcp * 2 + half
                        nc.tensor.matmul(
                            ps[:, half * 512:(half + 1) * 512],
                            xc[:, lc * 128:(lc + 1) * 128],
                            w1c[:],
                            start=True, stop=True,
                        )
                    dst = ht8[:, lcp * 2:lcp * 2 + 2, :].rearrange(
                        "p c w -> p (c w)"
                    )
                    if lcp % 2 == 0:
                        nc.scalar.activation(dst, ps[:], AF.Relu)
                    else:
                        nc.vector.tensor_scalar_max(dst, ps[:], 0.0)
                # qt'[n, l] = sum_d M[d, n] ht[d, l]
                qt = qp.tile([128, 4, L], bf16, tag="qT")
                for nck in range(4):
                    ps = pmm.tile([128, 1024], f32, tag="pmm")
                    for lg in range(2):
                        for dk in range(4):
                            nc.tensor.matmul(
                                ps[:, lg * 512:(lg + 1) * 512],
                                mm[:, dk, nck * 128:(nck + 1) * 128],
                                ht[:, dk, lg * 512:(lg + 1) * 512],
                                start=(dk == 0), stop=(dk == 3),
                            )
                    if nck == 3:
                        # split the last copy across both engines: scores(lc=0)
                        # stalls on exactly this tile
                        nc.scalar.copy(qt[:, nck, 0:512], ps[:, 0:512])
                        nc.vector.tensor_copy(qt[:, nck, 512:1024], ps[:, 512:1024])
                    elif nck % 2 == 0:
                        nc.scalar.copy(qt[:, nck, :], ps[:])
                    else:
                        nc.vector.tensor_copy(qt[:, nck, :], ps[:])
                state[e] = (ht, ht8, qt)

            def finalsum(e):
                """Sum the 9 scattered tap rows on PE, add b2, DMA out."""
                acc1 = msc.tile([1, L], f32, tag="acc1")
                psf = pmm.tile([1, 1024], f32, tag="pmm", name="psf")
                for lg in range(2):
                    sl = slice(lg * 512, (lg + 1) * 512)
                    nc.tensor.matmul(
                        psf[:, sl], oncb[0:9, 0:1], p9sh[0:9, e, sl],
                        start=True, stop=True,
                    )
                nc.scalar.activation(
                    acc1[:], psf[:], AF.Identity, bias=b2t[0:1, 0:1]
                )
                nc.sync.dma_start(o_out.ap()[e:e + 1, :], acc1[0:1, :])

            def attention(e):
                ht, ht8, qt = state[e]
                if STAGE < 2:
                    if e + 1 < E:
                        conv1_proj(e + 1)
                    return
                # ---- scores in M-layout; exp with fused -max bias and rowsum;
                #      normalize by 1/rowsum (per-partition scale) and
                #      PE-transpose each 128x128 prob tile into T-layout ----
                nmcol = msc.tile([128, 8], f32, tag="nmcol")
                rscol = msc.tile([128, 8], f32, tag="rscol")
                rcol = msc.tile([128, 8], f32, tag="rcol")
                et = ep.tile([128, 8, L], fp8, tag="eT")
                for lc in range(8):
                    ps = pmm.tile([128, 1024], f32, tag="pmm")
                    for mg in range(2):
                        for ncx in range(4):
                            nc.tensor.matmul(
                                ps[:, mg * 512:(mg + 1) * 512],
                                qt[:, ncx, lc * 128:(lc + 1) * 128],
                                ht[:, ncx, mg * 512:(mg + 1) * 512],
                                start=(ncx == 0), stop=(ncx == 3),
                            )
                    nc.vector.tensor_reduce(
                        nmcol[:, lc:lc + 1], ps[:], axis=AX.X, op=ALU.max, negate=True
                    )
                    expm = xm.tile([128, 1024], bf16, tag="expM")
                    nc.scalar.activation(
                        expm[:], ps[:], AF.Exp,
                        bias=nmcol[:, lc:lc + 1],
                        accum_out=rscol[:, lc:lc + 1],
                    )
                    nc.vector.reciprocal(rcol[:, lc:lc + 1], rscol[:, lc:lc + 1])
                    expn = xn.tile([128, 1024], bf16, tag="expN")
                    if lc % 2 == 0:
                        nc.scalar.activation(
                            expn[:], expm[:], AF.Identity,
                            scale=rcol[:, lc:lc + 1],
                        )
                    else:
                        nc.vector.tensor_scalar(
                            expn[:], expm[:], rcol[:, lc:lc + 1], None, ALU.mult
                        )
                    ptr = ptp.tile([128, 1024], bf16, tag="ptr")
                    for mc in range(8):
                        nc.tensor.transpose(
                            ptr[:, mc * 128:(mc + 1) * 128],
                            expn[:, mc * 128:(mc + 1) * 128],
                            identb[:],
                        )
                    for mc in range(0, 8, 2):
                        dst = et[:, mc:mc + 2, lc * 128:(lc + 1) * 128]
                        srcp = ptr[:, mc * 128:(mc + 2) * 128].rearrange(
                            "p (c w) -> p c w", c=2
                        )
                        if mc % 4 == 0:
                            nc.scalar.copy(dst, srcp)
                        else:
                            nc.vector.tensor_copy(dst, srcp)

                # final tap-sum of the previous element: the gpsimd/sync scatter
                # for it finished during the scores block, so PE never stalls
                if e > 0 and STAGE >= 8:
                    finalsum(e - 1)

                if STAGE < 4:
                    if e + 1 < E:
                        conv1_proj(e + 1)
                    return
                # ---- A[d, q] = sum_k hT8[k, d] Enorm^T[k, q]  (fp8 DoubleRow) ----
                # PSUM "start" zeroes a whole 2KB bank, so the four concurrent
                # 256-col accumulation groups must each own a distinct bank:
                # qg 0/2 -> psx banks 0/1, qg 1/3 -> psy banks 0/1.
                for dc in range(4):
                    psx = pmm.tile([128, 1024], f32, tag="pmm", name="psxA")
                    psy = pmm.tile([128, 1024], f32, tag="pmm", name="psyA")
                    for kc in range(0, 8, 2):
                        for qg in range(4):
                            ps = psx if qg % 2 == 0 else psy
                            col = (qg // 2) * 512
                            nc.tensor.matmul(
                                ps[:, col:col + 256],
                                ht8[:, kc:kc + 2, dc * 128:(dc + 1) * 128],
                                et[:, kc:kc + 2, qg * 256:(qg + 1) * 256],
                                start=(kc == 0), stop=(kc == 6),
                                perf_mode=DR,
                            )
                    for src, qoff in ((psx, 0), (psy, 256)):
                        s = src[:, :].rearrange("p (g w) -> p g w", g=2)[:, :, 0:256]
                        d = ht[:, 4 + dc, :].rearrange(
                            "p (g w) -> p g w", g=2
                        )[:, :, qoff:qoff + 256]
                        if qoff == 0:
                            nc.scalar.copy(d, s)
                        else:
                            nc.vector.tensor_copy(d, s)

                # next elem's prologue fills PE while conv2's DVE/DMA tail runs
                if e + 1 < E:
                    conv1_proj(e + 1)
                if STAGE < 6:
                    return
                # ---- conv2 taps: one 8-step chain over [H | A] chunks ----
                p9e = msc.tile([9, L], bf16, tag="p9e")
                p9ho = pmm.tile([9, 1024], f32, tag="pmm", name="p9ho")
                for lg in range(2):
                    sl = slice(lg * 512, (lg + 1) * 512)
                    for ck in range(8):
                        nc.tensor.matmul(
                            p9ho[:, sl], w2f[:, ck, :], ht[:, ck, sl],
                            start=(ck == 0), stop=(ck == 7),
                        )
                nc.scalar.copy(p9e[:], p9ho[:])
                if STAGE < 7:
                    return
                # scatter each tap row into its shifted, clipped window, split
                # across the gpsimd and sync DMA queues (byte-addressed, so the
                # unaligned partition bases are fine)
                for j, (dy, dx) in enumerate(_TAPS):
                    r0, r1 = max(0, 1 - dy), min(IMG, IMG + 1 - dy)
                    c0, c1 = max(0, 1 - dx), min(IMG, IMG + 1 - dx)
                    srcw = p9e[j:j + 1, :].rearrange("o (r w) -> o r w", w=IMG)[
                        :, r0 + dy - 1:r1 + dy - 1, c0 + dx - 1:c1 + dx - 1
                    ]
                    dstw = p9sh[j:j + 1, e, :].rearrange("o (r w) -> o r w", w=IMG)[
                        :, r0:r1, c0:c1
                    ]
                    if j % 3 == 0:
                        nc.gpsimd.dma_start(dstw, srcw)
                    else:
                        nc.sync.dma_start(dstw, srcw)

            conv1_proj(0)
            for e in range(E):
                attention(e)
            if STAGE >= 8:
                finalsum(E - 1)

    nc.compile()
    return nc


def _host_prep(x, W1, b1, Q, K, V, W2, b2):
    B = x.shape[0] * x.shape[1]
    xf = np.ascontiguousarray(x, np.float32).reshape(B, IMG, IMG)
    xpad = np.zeros((B, IMG + 2, IMG + 2), np.float32)
    xpad[:, 1:-1, 1:-1] = xf
    xcol = np.empty((B, 10, L), np.float32)
    for j, (dy, dx) in enumerate(_TAPS):
        xcol[:, j] = xpad[:, dy:dy + IMG, dx:dx + IMG].reshape(B, L)
    xcol[:, 9] = 1.0
    import ml_dtypes
    xcol = xcol.astype(ml_dtypes.bfloat16)
    w1r = np.asarray(W1, np.float32).reshape(P, 9)
    w1cb = np.ascontiguousarray(
        np.concatenate([w1r.T, np.asarray(b1, np.float32)[None, :]], axis=0)
    ).astype(ml_dtypes.bfloat16)  # [10, P]
    M = np.asarray(Q, np.float32) @ np.asarray(K, np.float32).T
    mm = np.ascontiguousarray(M.reshape(4, 128, P).transpose(1, 0, 2)).astype(
        ml_dtypes.bfloat16
    )
    w2r = np.asarray(W2, np.float32).reshape(P, 9)
    w2h = w2r.reshape(4, 128, 9).transpose(1, 0, 2)       # [128, 4, 9]
    w2vt_f = np.asarray(V, np.float32) @ w2r              # [512 d, 9 tap]
    w2v = w2vt_f.reshape(4, 128, 9).transpose(1, 0, 2)    # [128, 4, 9]
    w2m = np.ascontiguousarray(
        np.concatenate([w2h, w2v], axis=1)
    ).astype(ml_dtypes.bfloat16)                          # [128, 8, 9]
    b2v = np.asarray(b2, np.float32).reshape(1, 1)
    in_maps = []
    for c in range(NCORES):
        in_maps.append({
            "xcol": np.ascontiguousarray(xcol[E * c:E * (c + 1)]),
            "W1cb": w1cb, "Mm": mm, "W2m": w2m, "b2v": b2v,
        })
    return in_maps


def kernel(x, W1, b1, Q, K, V, W2, b2):
    from concourse.bass_utils import run_bass_kernel_spmd

    in_maps = _host_prep(x, W1, b1, Q, K, V, W2, b2)
    if "nc" not in _built:
        _built["nc"] = _build_nc()
    nc = _built["nc"]
    res = run_bass_kernel_spmd(nc, in_maps, core_ids=list(range(NCORES)))
    full = np.concatenate([res.results[c]["out"] for c in range(NCORES)], axis=0)
    return np.ascontiguousarray(
        full.reshape(x.shape[0], x.shape[1], IMG, IMG).astype(np.float32)
    )


# revision 21
# speedup vs baseline: 1.0195x; 1.0018x over previous
"""Self-contained Trainium2 kernel for nn_BanzhafModule (conv1 -> self-attention -> conv2).

Data-parallel over 8 NeuronCores: each core processes 4 of the 32 (b*a) batch
elements end-to-end; no collectives.

Algebraic folds (host-side, exact):
  * S = (hx Q)(hx K)^T = hx (Q K^T) hx^T  -> precompute M = Q K^T; the K
    projection disappears (scores contract qt' = M^T hx^T against ht directly).
  * o enters the output only through conv2:  p9o = W2^T o^T
    = (W2^T V^T) (hx^T E^T)  -> precompute W2V = V @ W2r; the V projection and
    the O matmul are replaced by A = hx^T E^T (fp8 DoubleRow) and a tiny
    [9 x L] contraction.

Precision: conv1 / qt' / scores / p9h in fp32r; exp probs (normalized by the
rowsum before the transpose, as a per-partition scale in M-layout) + token-major
h in fp8e4; A copy + p9oA in bf16; taps in bf16.  b1 rides in the conv as a
10th "ones" tap row.  PSUM accumulation groups never share a 2KB bank.
"""

import numpy as np

E = 4          # batch elements per core
NCORES = 8
IMG = 32       # t = v = 32
L = IMG * IMG  # 1024 tokens
P = 512        # planes

_TAPS = [(dy, dx) for dy in range(3) for dx in range(3)]

_built = {}


def _build_nc():
    import os
    STAGE = int(os.environ.get("KSTAGE", "99"))
    import concourse.mybir as mybir
    from concourse import bacc
    from concourse.tile import TileContext
    from concourse.masks import make_identity

    f32, f32r, bf16 = mybir.dt.float32, mybir.dt.float32r, mybir.dt.bfloat16
    fp8 = mybir.dt.float8e4
    AF = mybir.ActivationFunctionType
    ALU = mybir.AluOpType
    AX = mybir.AxisListType
    DR = mybir.MatmulPerfMode.DoubleRow

    nc = bacc.Bacc("TRN2", target_bir_lowering=False, debug=False, num_devices=NCORES)

    i_xcol = nc.dram_tensor("xcol", [E, 10, L], bf16, kind="ExternalInput")
    i_w1 = nc.dram_tensor("W1cb", [10, P], bf16, kind="ExternalInput")
    i_m = nc.dram_tensor("Mm", [128, 4, P], bf16, kind="ExternalInput")
    i_w2 = nc.dram_tensor("W2m", [128, 8, 9], bf16, kind="ExternalInput")
    i_b2 = nc.dram_tensor("b2v", [1, 1], f32, kind="ExternalInput")
    o_out = nc.dram_tensor("out", [E, L], f32, kind="ExternalOutput")

    ones_col_d = nc.inline_tensor(np.ones((128, 1), np.float32), name="ones_col")

    with TileContext(nc) as tc:
        with (
            tc.tile_pool(name="wts", bufs=1) as wts,
            tc.tile_pool(name="hp", bufs=2) as hp,
            tc.tile_pool(name="htp", bufs=2) as htp,
            tc.tile_pool(name="qp", bufs=2) as qp,
            tc.tile_pool(name="ep", bufs=2) as ep,
            tc.tile_pool(name="xp", bufs=2) as xp,
            tc.tile_pool(name="msc", bufs=2) as msc,
            tc.tile_pool(name="fin", bufs=1) as fin,
            tc.tile_pool(name="pmm", bufs=3, space="PSUM") as pmm,
            tc.tile_pool(name="ptp", bufs=2, space="PSUM") as ptp,
            tc.tile_pool(name="xm", bufs=2) as xm,
            tc.tile_pool(name="xn", bufs=2) as xn,
        ):
            # ---- weights / constants (persistent, DMA'd straight in) ----
            prefetch = {}
            xcf0 = xp.tile([10, L], bf16, tag="xcol", name="xcf0")
            nc.sync.dma_start(xcf0[:], i_xcol.ap()[0])
            prefetch[0] = xcf0

            w1c = wts.tile([10, P], bf16, tag="w1c")
            nc.gpsimd.dma_start(w1c[:], i_w1.ap())
            mm = wts.tile([128, 4, P], bf16, tag="mm", name="mm")
            nc.sync.dma_start(mm[:], i_m.ap())
            w2f = wts.tile([128, 8, 9], bf16, tag="w2f")
            nc.sync.dma_start(w2f[:], i_w2.ap())

            onc = wts.tile([128, 1], f32)
            nc.sync.dma_start(onc[:], ones_col_d.ap())
            oncb = wts.tile([128, 1], bf16)
            nc.vector.tensor_copy(oncb[:], onc[:])
            identb = wts.tile([128, 128], bf16)
            make_identity(nc, identb[:])

            b2t = wts.tile([1, 1], f32)
            nc.sync.dma_start(b2t[:], i_b2.ap())
            p9sh = fin.tile([9, E, L], bf16)
            nc.gpsimd.memset(p9sh[:], 0.0)

            state = {}

            def conv1_proj(e):
                """conv1 (both layouts) + qt' = M^T hx^T projection."""
                xc = prefetch.pop(e, None)
                if xc is None:
                    xc = xp.tile([10, L], bf16, tag="xcol")
                    nc.sync.dma_start(xc[:], i_xcol.ap()[e])
                # channel-major: ht[p, l] = relu(sum_j w1cb[j, p] xc10[j, l])
                ht = hp.tile([128, 8, L], bf16, tag="H")
                for ck in range(4):
                    ps = pmm.tile([128, 1024], f32, tag="pmm")
                    for lg in range(2):
                        nc.tensor.matmul(
                            ps[:, lg * 512:(lg + 1) * 512],
                            w1c[:, ck * 128:(ck + 1) * 128],
                            xc[:, lg * 512:(lg + 1) * 512],
                            start=True, stop=True,
                        )
                    if ck % 2 == 0:
                        nc.scalar.activation(ht[:, ck, :], ps[:], AF.Relu)
                    else:
                        nc.vector.tensor_scalar_max(ht[:, ck, :], ps[:], 0.0)
                # token-major: hT8[l, lc, p] = relu(sum_j xc10[j, l] w1cb[j, p]) in fp8
                ht8 = htp.tile([128, 8, P], fp8, tag="HT8")
                for lcp in range(4):
                    ps = pmm.tile([128, 1024], f32, tag="pmm")
                    for half in range(2):
                        lc = lcp * 2 + half
                        nc.tensor.matmul(
                            ps[:, half * 512:(half + 1) * 512],
                            xc[:, lc * 128:(lc + 1) * 128],
                            w1c[:],
                            start=True, stop=True,
                        )
                    dst = ht8[:, lcp * 2:lcp * 2 + 2, :].rearrange(
                        "p c w -> p (c w)"
                    )
                    if lcp % 2 == 0:
                        nc.scalar.activation(dst, ps[:], AF.Relu)
                    else:
                        nc.vector.tensor_scalar_max(dst, ps[:], 0.0)
                # qt'[n, l] = sum_d M[d, n] ht[d, l]
                qt = qp.tile([128, 4, L], bf16, tag="qT")
                for nck in range(4):
                    ps = pmm.tile([128, 1024], f32, tag="pmm")
                    for lg in range(2):
                        for dk in range(4):
                            nc.tensor.matmul(
                                ps[:, lg * 512:(lg + 1) * 512],
                                mm[:, dk, nck * 128:(nck + 1) * 128],
                                ht[:, dk, lg * 512:(lg + 1) * 512],
                                start=(dk == 0), stop=(dk == 3),
                            )
                    if nck % 2 == 0:
                        nc.scalar.copy(qt[:, nck, :], ps[:])
                    else:
                        nc.vector.tensor_copy(qt[:, nck, :], ps[:])
                state[e] = (ht, ht8, qt)

            def finalsum(e):
                """Sum the 9 scattered tap rows on PE, add b2, DMA out."""
                acc1 = msc.tile([1, L], f32, tag="acc1")
                psf = pmm.tile([1, 1024], f32, tag="pmm", name="psf")
                for lg in range(2):
                    sl = slice(lg * 512, (lg + 1) * 512)
                    nc.tensor.matmul(
                        psf[:, sl], oncb[0:9, 0:1], p9sh[0:9, e, sl],
                        start=True, stop=True,
                    )
                nc.scalar.activation(
                    acc1[:], psf[:], AF.Identity, bias=b2t[0:1, 0:1]
                )
                nc.sync.dma_start(o_out.ap()[e:e + 1, :], acc1[0:1, :])

            def attention(e):
                ht, ht8, qt = state[e]
                if STAGE < 2:
                    if e + 1 < E:
                        conv1_proj(e + 1)
                    return
                # ---- scores in M-layout; exp with fused -max bias and rowsum;
                #      normalize by 1/rowsum (per-partition scale) and
                #      PE-transpose each 128x128 prob tile into T-layout ----
                nmcol = msc.tile([128, 8], f32, tag="nmcol")
                rscol = msc.tile([128, 8], f32, tag="rscol")
                rcol = msc.tile([128, 8], f32, tag="rcol")
                et = ep.tile([128, 8, L], fp8, tag="eT")
                for lc in range(8):
                    ps = pmm.tile([128, 1024], f32, tag="pmm")
                    for mg in range(2):
                        for ncx in range(4):
                            nc.tensor.matmul(
                                ps[:, mg * 512:(mg + 1) * 512],
                                qt[:, ncx, lc * 128:(lc + 1) * 128],
                                ht[:, ncx, mg * 512:(mg + 1) * 512],
                                start=(ncx == 0), stop=(ncx == 3),
                            )
                    nc.vector.tensor_reduce(
                        nmcol[:, lc:lc + 1], ps[:], axis=AX.X, op=ALU.max, negate=True
                    )
                    expm = xm.tile([128, 1024], bf16, tag="expM")
                    nc.scalar.activation(
                        expm[:], ps[:], AF.Exp,
                        bias=nmcol[:, lc:lc + 1],
                        accum_out=rscol[:, lc:lc + 1],
                    )
                    nc.vector.reciprocal(rcol[:, lc:lc + 1], rscol[:, lc:lc + 1])
                    expn = xn.tile([128, 1024], bf16, tag="expN")
                    if lc % 2 == 0:
                        nc.scalar.activation(
                            expn[:], expm[:], AF.Identity,
                            scale=rcol[:, lc:lc + 1],
                        )
                    else:
                        nc.vector.tensor_scalar(
                            expn[:], expm[:], rcol[:, lc:lc + 1], None, ALU.mult
                        )
                    ptr = ptp.tile([128, 1024], bf16, tag="ptr")
                    for mc in range(8):
                        nc.tensor.transpose(
                            ptr[:, mc * 128:(mc + 1) * 128],
                            expn[:, mc * 128:(mc + 1) * 128],
                            identb[:],
                        )
                    for mc in range(0, 8, 2):
                        dst = et[:, mc:mc + 2, lc * 128:(lc + 1) * 128]
                        srcp = ptr[:, mc * 128:(mc + 2) * 128].rearrange(
                            "p (c w) -> p c w", c=2
                        )
                        if mc % 4 == 0:
                            nc.scalar.copy(dst, srcp)
                        else:
                            nc.vector.tensor_copy(dst, srcp)

                # final tap-sum of the previous element: the gpsimd/sync scatter
                # for it finished during the scores block, so PE never stalls
                if e > 0 and STAGE >= 8:
                    finalsum(e - 1)

                if STAGE < 4:
                    if e + 1 < E:
                        conv1_proj(e + 1)
                    return
                # ---- A[d, q] = sum_k hT8[k, d] Enorm^T[k, q]  (fp8 DoubleRow) ----
                # PSUM "start" zeroes a whole 2KB bank, so the four concurrent
                # 256-col accumulation groups must each own a distinct bank:
                # qg 0/2 -> psx banks 0/1, qg 1/3 -> psy banks 0/1.
                for dc in range(4):
                    psx = pmm.tile([128, 1024], f32, tag="pmm", name="psxA")
                    psy = pmm.tile([128, 1024], f32, tag="pmm", name="psyA")
                    for kc in range(0, 8, 2):
                        for qg in range(4):
                            ps = psx if qg % 2 == 0 else psy
                            col = (qg // 2) * 512
                            nc.tensor.matmul(
                                ps[:, col:col + 256],
                                ht8[:, kc:kc + 2, dc * 128:(dc + 1) * 128],
                                et[:, kc:kc + 2, qg * 256:(qg + 1) * 256],
                                start=(kc == 0), stop=(kc == 6),
                                perf_mode=DR,
                            )
                    for src, qoff in ((psx, 0), (psy, 256)):
                        s = src[:, :].rearrange("p (g w) -> p g w", g=2)[:, :, 0:256]
                        d = ht[:, 4 + dc, :].rearrange(
                            "p (g w) -> p g w", g=2
                        )[:, :, qoff:qoff + 256]
                        if qoff == 0:
                            nc.scalar.copy(d, s)
                        else:
                            nc.vector.tensor_copy(d, s)

                # next elem's prologue fills PE while conv2's DVE/DMA tail runs
                if e + 1 < E:
                    conv1_proj(e + 1)
                if STAGE < 6:
                    return
                # ---- conv2 taps: one 8-step chain over [H | A] chunks ----
                p9e = msc.tile([9, L], bf16, tag="p9e")
                p9ho = pmm.tile([9, 1024], f32, tag="pmm", name="p9ho")
                for lg in range(2):
                    sl = slice(lg * 512, (lg + 1) * 512)
                    for ck in range(8):
                        nc.tensor.matmul(
                            p9ho[:, sl], w2f[:, ck, :], ht[:, ck, sl],
                            start=(ck == 0), stop=(ck == 7),
                        )
                nc.scalar.copy(p9e[:], p9ho[:])
                if STAGE < 7:
                    return
                # scatter each tap row into its shifted, clipped window, split
                # across the gpsimd and sync DMA queues (byte-addressed, so the
                # unaligned partition bases are fine)
                for j, (dy, dx) in enumerate(_TAPS):
                    r0, r1 = max(0, 1 - dy), min(IMG, IMG + 1 - dy)
                    c0, c1 = max(0, 1 - dx), min(IMG, IMG + 1 - dx)
                    srcw = p9e[j:j + 1, :].rearrange("o (r w) -> o r w", w=IMG)[
                        :, r0 + dy - 1:r1 + dy - 1, c0 + dx - 1:c1 + dx - 1
                    ]
                    dstw = p9sh[j:j + 1, e, :].rearrange("o (r w) -> o r w", w=IMG)[
                        :, r0:r1, c0:c1
                    ]
                    if j % 3 == 0:
                        nc.gpsimd.dma_start(dstw, srcw)
                    else:
                        nc.sync.dma_start(dstw, srcw)

            conv1_proj(0)
            for e in range(E):
                attention(e)
            if STAGE >= 8:
                finalsum(E - 1)

    nc.compile()
    return nc


def _host_prep(x, W1, b1, Q, K, V, W2, b2):
    B = x.shape[0] * x.shape[1]
    xf = np.ascontiguousarray(x, np.float32).reshape(B, IMG, IMG)
    xpad = np.zeros((B, IMG + 2, IMG + 2), np.float32)
    xpad[:, 1:-1, 1:-1] = xf
    xcol = np.empty((B, 10, L), np.float32)
    for j, (dy, dx) in enumerate(_TAPS):
        xcol[:, j] = xpad[:, dy:dy + IMG, dx:dx + IMG].reshape(B, L)
    xcol[:, 9] = 1.0
    import ml_dtypes
    xcol = xcol.astype(ml_dtypes.bfloat16)
    w1r = np.asarray(W1, np.float32).reshape(P, 9)
    w1cb = np.ascontiguousarray(
        np.concatenate([w1r.T, np.asarray(b1, np.float32)[None, :]], axis=0)
    ).astype(ml_dtypes.bfloat16)  # [10, P]
    M = np.asarray(Q, np.float32) @ np.asarray(K, np.float32).T
    mm = np.ascontiguousarray(M.reshape(4, 128, P).transpose(1, 0, 2)).astype(
        ml_dtypes.bfloat16
    )
    w2r = np.asarray(W2, np.float32).reshape(P, 9)
    w2h = w2r.reshape(4, 128, 9).transpose(1, 0, 2)       # [128, 4, 9]
    w2vt_f = np.asarray(V, np.float32) @ w2r              # [512 d, 9 tap]
    w2v = w2vt_f.reshape(4, 128, 9).transpose(1, 0, 2)    # [128, 4, 9]
    w2m = np.ascontiguousarray(
        np.concatenate([w2h, w2v], axis=1)
    ).astype(ml_dtypes.bfloat16)                          # [128, 8, 9]
    b2v = np.asarray(b2, np.float32).reshape(1, 1)
    in_maps = []
    for c in range(NCORES):
        in_maps.append({
            "xcol": np.ascontiguousarray(xcol[E * c:E * (c + 1)]),
            "W1cb": w1cb, "Mm": mm, "W2m": w2m, "b2v": b2v,
        })
    return in_maps


def kernel(x, W1, b1, Q, K, V, W2, b2):
    from concourse.bass_utils import run_bass_kernel_spmd

    in_maps = _host_prep(x, W1, b1, Q, K, V, W2, b2)
    if "nc" not in _built:
        _built["nc"] = _build_nc()
    nc = _built["nc"]
    res = run_bass_kernel_spmd(nc, in_maps, core_ids=list(range(NCORES)))
    full = np.concatenate([res.results[c]["out"] for c in range(NCORES)], axis=0)
    return np.ascontiguousarray(
        full.reshape(x.shape[0], x.shape[1], IMG, IMG).astype(np.float32)
    )


# revision 23
# speedup vs baseline: 1.0355x; 1.0157x over previous
"""Self-contained Trainium2 kernel for nn_BanzhafModule (conv1 -> self-attention -> conv2).

Data-parallel over 8 NeuronCores: each core processes 4 of the 32 (b*a) batch
elements end-to-end; no collectives.

Algebraic folds (host-side, exact):
  * S = (hx Q)(hx K)^T = hx (Q K^T) hx^T  -> precompute M = Q K^T; the K
    projection disappears (scores contract qt' = M^T hx^T against ht directly).
  * o enters the output only through conv2:  p9o = W2^T o^T
    = (W2^T V^T) (hx^T E^T)  -> precompute W2V = V @ W2r; the V projection and
    the O matmul are replaced by A = hx^T E^T (fp8 DoubleRow) and a tiny
    [9 x L] contraction.

Precision: conv1 / qt' / scores / p9h in fp32r; exp probs (normalized by the
rowsum before the transpose, as a per-partition scale in M-layout) + token-major
h in fp8e4; A copy + p9oA in bf16; taps in bf16.  b1 rides in the conv as a
10th "ones" tap row.  PSUM accumulation groups never share a 2KB bank.
"""

import numpy as np

E = 4          # batch elements per core
NCORES = 8
IMG = 32       # t = v = 32
L = IMG * IMG  # 1024 tokens
P = 512        # planes

_TAPS = [(dy, dx) for dy in range(3) for dx in range(3)]

_built = {}


def _build_nc():
    import os
    STAGE = int(os.environ.get("KSTAGE", "99"))
    import concourse.mybir as mybir
    from concourse import bacc
    from concourse.tile import TileContext
    from concourse.masks import make_identity

    f32, f32r, bf16 = mybir.dt.float32, mybir.dt.float32r, mybir.dt.bfloat16
    fp8 = mybir.dt.float8e4
    AF = mybir.ActivationFunctionType
    ALU = mybir.AluOpType
    AX = mybir.AxisListType
    DR = mybir.MatmulPerfMode.DoubleRow

    nc = bacc.Bacc("TRN2", target_bir_lowering=False, debug=False, num_devices=NCORES)

    i_xcol = nc.dram_tensor("xcol", [E, 10, L], bf16, kind="ExternalInput")
    i_w1 = nc.dram_tensor("W1cb", [10, P], bf16, kind="ExternalInput")
    i_m = nc.dram_tensor("Mm", [128, 4, P], bf16, kind="ExternalInput")
    i_w2 = nc.dram_tensor("W2m", [128, 8, 9], bf16, kind="ExternalInput")
    i_b2 = nc.dram_tensor("b2v", [1, 1], f32, kind="ExternalInput")
    o_out = nc.dram_tensor("out", [E, L], f32, kind="ExternalOutput")

    ones_col_d = nc.inline_tensor(np.ones((128, 1), np.float32), name="ones_col")

    with TileContext(nc) as tc:
        with (
            tc.tile_pool(name="wts", bufs=1) as wts,
            tc.tile_pool(name="hp", bufs=2) as hp,
            tc.tile_pool(name="htp", bufs=2) as htp,
            tc.tile_pool(name="qp", bufs=2) as qp,
            tc.tile_pool(name="ep", bufs=2) as ep,
            tc.tile_pool(name="xp", bufs=2) as xp,
            tc.tile_pool(name="msc", bufs=2) as msc,
            tc.tile_pool(name="fin", bufs=1) as fin,
            tc.tile_pool(name="pmm", bufs=3, space="PSUM") as pmm,
            tc.tile_pool(name="ptp", bufs=2, space="PSUM") as ptp,
            tc.tile_pool(name="xm", bufs=2) as xm,
            tc.tile_pool(name="xn", bufs=2) as xn,
        ):
            # ---- weights / constants (persistent, DMA'd straight in) ----
            prefetch = {}
            xcf0 = xp.tile([10, L], bf16, tag="xcol", name="xcf0")
            nc.sync.dma_start(xcf0[:], i_xcol.ap()[0])
            prefetch[0] = xcf0

            w1c = wts.tile([10, P], bf16, tag="w1c")
            nc.gpsimd.dma_start(w1c[:], i_w1.ap())
            mm = wts.tile([128, 4, P], bf16, tag="mm", name="mm")
            nc.sync.dma_start(mm[:], i_m.ap())
            w2f = wts.tile([128, 8, 9], bf16, tag="w2f")
            nc.sync.dma_start(w2f[:], i_w2.ap())

            onc = wts.tile([128, 1], f32)
            nc.sync.dma_start(onc[:], ones_col_d.ap())
            oncb = wts.tile([128, 1], bf16)
            nc.vector.tensor_copy(oncb[:], onc[:])
            identb = wts.tile([128, 128], bf16)
            make_identity(nc, identb[:])

            b2t = wts.tile([1, 1], f32)
            nc.sync.dma_start(b2t[:], i_b2.ap())
            p9sh = fin.tile([9, E, L], bf16)
            nc.gpsimd.memset(p9sh[:], 0.0)

            state = {}

            def conv1_proj(e):
                """conv1 (both layouts) + qt' = M^T hx^T projection."""
                xc = prefetch.pop(e, None)
                if xc is None:
                    xc = xp.tile([10, L], bf16, tag="xcol")
                    nc.sync.dma_start(xc[:], i_xcol.ap()[e])
                # channel-major: ht[p, l] = relu(sum_j w1cb[j, p] xc10[j, l])
                ht = hp.tile([128, 8, L], bf16, tag="H")
                for ck in range(4):
                    ps = pmm.tile([128, 1024], f32, tag="pmm")
                    for lg in range(2):
                        nc.tensor.matmul(
                            ps[:, lg * 512:(lg + 1) * 512],
                            w1c[:, ck * 128:(ck + 1) * 128],
                            xc[:, lg * 512:(lg + 1) * 512],
                            start=True, stop=True,
                        )
                    if ck % 2 == 0:
                        nc.scalar.activation(ht[:, ck, :], ps[:], AF.Relu)
                    else:
                        nc.vector.tensor_scalar_max(ht[:, ck, :], ps[:], 0.0)
                # token-major: hT8[l, lc, p] = relu(sum_j xc10[j, l] w1cb[j, p]) in fp8
                ht8 = htp.tile([128, 8, P], fp8, tag="HT8")
                for lcp in range(4):
                    ps = pmm.tile([128, 1024], f32, tag="pmm")
                    for half in range(2):
                        lc = lcp * 2 + half
                        nc.tensor.matmul(
                            ps[:, half * 512:(half + 1) * 512],
                            xc[:, lc * 128:(lc + 1) * 128],
                            w1c[:],
                            start=True, stop=True,
                        )
                    dst = ht8[:, lcp * 2:lcp * 2 + 2, :].rearrange(
                        "p c w -> p (c w)"
                    )
                    if lcp % 2 == 0:
                        nc.scalar.activation(dst, ps[:], AF.Relu)
                    else:
                        nc.vector.tensor_scalar_max(dst, ps[:], 0.0)
                # qt'[n, l] = sum_d M[d, n] ht[d, l]
                qt = qp.tile([128, 4, L], bf16, tag="qT")
                for nck in range(4):
                    ps = pmm.tile([128, 1024], f32, tag="pmm")
                    for lg in range(2):
                        for dk in range(4):
                            nc.tensor.matmul(
                                ps[:, lg * 512:(lg + 1) * 512],
                                mm[:, dk, nck * 128:(nck + 1) * 128],
                                ht[:, dk, lg * 512:(lg + 1) * 512],
                                start=(dk == 0), stop=(dk == 3),
                            )
                    if nck % 2 == 0:
                        nc.scalar.copy(qt[:, nck, :], ps[:])
                    else:
                        nc.vector.tensor_copy(qt[:, nck, :], ps[:])
                state[e] = (ht, ht8, qt)

            def finalsum(e):
                """Sum the 9 scattered tap rows on PE, add b2, DMA out."""
                acc1 = msc.tile([1, L], f32, tag="acc1")
                psf = pmm.tile([1, 1024], f32, tag="pmm", name="psf")
                for lg in range(2):
                    sl = slice(lg * 512, (lg + 1) * 512)
                    nc.tensor.matmul(
                        psf[:, sl], oncb[0:9, 0:1], p9sh[0:9, e, sl],
                        start=True, stop=True,
                    )
                nc.scalar.activation(
                    acc1[:], psf[:], AF.Identity, bias=b2t[0:1, 0:1]
                )
                nc.sync.dma_start(o_out.ap()[e:e + 1, :], acc1[0:1, :])

            def attention(e):
                ht, ht8, qt = state[e]
                if STAGE < 2:
                    if e + 1 < E:
                        conv1_proj(e + 1)
                    return
                # ---- scores in M-layout; exp with fused -max bias and rowsum;
                #      normalize by 1/rowsum (per-partition scale) and
                #      PE-transpose each 128x128 prob tile into T-layout ----
                nmcol = msc.tile([128, 8], f32, tag="nmcol")
                rscol = msc.tile([128, 8], f32, tag="rscol")
                rcol = msc.tile([128, 8], f32, tag="rcol")
                et = ep.tile([128, 8, L], fp8, tag="eT")
                for lc in range(8):
                    ps = pmm.tile([128, 1024], f32, tag="pmm")
                    for mg in range(2):
                        for ncx in range(4):
                            nc.tensor.matmul(
                                ps[:, mg * 512:(mg + 1) * 512],
                                qt[:, ncx, lc * 128:(lc + 1) * 128],
                                ht[:, ncx, mg * 512:(mg + 1) * 512],
                                start=(ncx == 0), stop=(ncx == 3),
                            )
                    nc.vector.tensor_reduce(
                        nmcol[:, lc:lc + 1], ps[:], axis=AX.X, op=ALU.max, negate=True
                    )
                    expm = xm.tile([128, 1024], bf16, tag="expM")
                    nc.scalar.activation(
                        expm[:], ps[:], AF.Exp,
                        bias=nmcol[:, lc:lc + 1],
                        accum_out=rscol[:, lc:lc + 1],
                    )
                    nc.vector.reciprocal(rcol[:, lc:lc + 1], rscol[:, lc:lc + 1])
                    expn = xn.tile([128, 1024], bf16, tag="expN")
                    if lc % 2 == 0:
                        nc.scalar.activation(
                            expn[:], expm[:], AF.Identity,
                            scale=rcol[:, lc:lc + 1],
                        )
                    else:
                        nc.vector.tensor_scalar(
                            expn[:], expm[:], rcol[:, lc:lc + 1], None, ALU.mult
                        )
                    ptr = ptp.tile([128, 1024], bf16, tag="ptr")
                    for mc in range(8):
                        nc.tensor.transpose(
                            ptr[:, mc * 128:(mc + 1) * 128],
                            expn[:, mc * 128:(mc + 1) * 128],
                            identb[:],
                        )
                    for mc in range(0, 8, 2):
                        dst = et[:, mc:mc + 2, lc * 128:(lc + 1) * 128]
                        srcp = ptr[:, mc * 128:(mc + 2) * 128].rearrange(
                            "p (c w) -> p c w", c=2
                        )
                        if mc % 4 == 0:
                            nc.scalar.copy(dst, srcp)
                        else:
                            nc.vector.tensor_copy(dst, srcp)

                # final tap-sum of the previous element: the gpsimd/sync scatter
                # for it finished during the scores block, so PE never stalls
                if e > 0 and STAGE >= 8:
                    finalsum(e - 1)

                if STAGE < 4:
                    if e + 1 < E:
                        conv1_proj(e + 1)
                    return
                # ---- A[d, q] = sum_k hT8[k, d] Enorm^T[k, q]  (fp8 DoubleRow) ----
                # PSUM "start" zeroes a whole 2KB bank, so the four concurrent
                # 256-col accumulation groups must each own a distinct bank:
                # qg 0/2 -> psx banks 0/1, qg 1/3 -> psy banks 0/1.
                # conv2 tap chains open early on the H chunks; each A chunk's
                # tap matmul is appended as soon as that chunk lands, so the
                # epilogue after the A loop is just two tap matmuls + copies
                p9e = msc.tile([9, L], bf16, tag="p9e")
                p9ho = [
                    ptp.tile([9, 512], f32, tag="ptr", name=f"p9ho{lg}")
                    for lg in range(2)
                ]
                for lg in range(2):
                    sl = slice(lg * 512, (lg + 1) * 512)
                    for ck in range(4):
                        nc.tensor.matmul(
                            p9ho[lg][:], w2f[:, ck, :], ht[:, ck, sl],
                            start=(ck == 0), stop=False,
                        )
                for dc in range(4):
                    psx = pmm.tile([128, 1024], f32, tag="pmm", name="psxA")
                    psy = pmm.tile([128, 1024], f32, tag="pmm", name="psyA")
                    for kc in range(0, 8, 2):
                        for qg in range(4):
                            ps = psx if qg % 2 == 0 else psy
                            col = (qg // 2) * 512
                            nc.tensor.matmul(
                                ps[:, col:col + 256],
                                ht8[:, kc:kc + 2, dc * 128:(dc + 1) * 128],
                                et[:, kc:kc + 2, qg * 256:(qg + 1) * 256],
                                start=(kc == 0), stop=(kc == 6),
                                perf_mode=DR,
                            )
                    # previous chunk's tap matmul: its copy has drained by now
                    if dc > 0:
                        for lg in range(2):
                            sl = slice(lg * 512, (lg + 1) * 512)
                            nc.tensor.matmul(
                                p9ho[lg][:], w2f[:, 3 + dc, :], ht[:, 3 + dc, sl],
                                start=False, stop=False,
                            )
                    for src, qoff in ((psx, 0), (psy, 256)):
                        s = src[:, :].rearrange("p (g w) -> p g w", g=2)[:, :, 0:256]
                        d = ht[:, 4 + dc, :].rearrange(
                            "p (g w) -> p g w", g=2
                        )[:, :, qoff:qoff + 256]
                        if qoff == 0:
                            nc.scalar.copy(d, s)
                        else:
                            nc.vector.tensor_copy(d, s)
                for lg in range(2):
                    sl = slice(lg * 512, (lg + 1) * 512)
                    nc.tensor.matmul(
                        p9ho[lg][:], w2f[:, 7, :], ht[:, 7, sl],
                        start=False, stop=True,
                    )

                if STAGE < 6:
                    if e + 1 < E:
                        conv1_proj(e + 1)
                    return
                nc.scalar.copy(p9e[:, 0:512], p9ho[0][:])
                nc.vector.tensor_copy(p9e[:, 512:1024], p9ho[1][:])

                # next elem's prologue fills PE while conv2's DVE/DMA tail runs
                if e + 1 < E:
                    conv1_proj(e + 1)
                if STAGE < 7:
                    return
                # scatter each tap row into its shifted, clipped window, split
                # across the gpsimd and sync DMA queues (byte-addressed, so the
                # unaligned partition bases are fine)
                for j, (dy, dx) in enumerate(_TAPS):
                    r0, r1 = max(0, 1 - dy), min(IMG, IMG + 1 - dy)
                    c0, c1 = max(0, 1 - dx), min(IMG, IMG + 1 - dx)
                    srcw = p9e[j:j + 1, :].rearrange("o (r w) -> o r w", w=IMG)[
                        :, r0 + dy - 1:r1 + dy - 1, c0 + dx - 1:c1 + dx - 1
                    ]
                    dstw = p9sh[j:j + 1, e, :].rearrange("o (r w) -> o r w", w=IMG)[
                        :, r0:r1, c0:c1
                    ]
                    if j % 3 == 0:
                        nc.gpsimd.dma_start(dstw, srcw)
                    else:
                        nc.sync.dma_start(dstw, srcw)

            conv1_proj(0)
            for e in range(E):
                attention(e)
            if STAGE >= 8:
                finalsum(E - 1)

    nc.compile()
    return nc


def _host_prep(x, W1, b1, Q, K, V, W2, b2):
    B = x.shape[0] * x.shape[1]
    xf = np.ascontiguousarray(x, np.float32).reshape(B, IMG, IMG)
    xpad = np.zeros((B, IMG + 2, IMG + 2), np.float32)
    xpad[:, 1:-1, 1:-1] = xf
    xcol = np.empty((B, 10, L), np.float32)
    for j, (dy, dx) in enumerate(_TAPS):
        xcol[:, j] = xpad[:, dy:dy + IMG, dx:dx + IMG].reshape(B, L)
    xcol[:, 9] = 1.0
    import ml_dtypes
    xcol = xcol.astype(ml_dtypes.bfloat16)
    w1r = np.asarray(W1, np.float32).reshape(P, 9)
    w1cb = np.ascontiguousarray(
        np.concatenate([w1r.T, np.asarray(b1, np.float32)[None, :]], axis=0)
    ).astype(ml_dtypes.bfloat16)  # [10, P]
    M = np.asarray(Q, np.float32) @ np.asarray(K, np.float32).T
    mm = np.ascontiguousarray(M.reshape(4, 128, P).transpose(1, 0, 2)).astype(
        ml_dtypes.bfloat16
    )
    w2r = np.asarray(W2, np.float32).reshape(P, 9)
    w2h = w2r.reshape(4, 128, 9).transpose(1, 0, 2)       # [128, 4, 9]
    w2vt_f = np.asarray(V, np.float32) @ w2r              # [512 d, 9 tap]
    w2v = w2vt_f.reshape(4, 128, 9).transpose(1, 0, 2)    # [128, 4, 9]
    w2m = np.ascontiguousarray(
        np.concatenate([w2h, w2v], axis=1)
    ).astype(ml_dtypes.bfloat16)                          # [128, 8, 9]
    b2v = np.asarray(b2, np.float32).reshape(1, 1)
    in_maps = []
    for c in range(NCORES):
        in_maps.append({
            "xcol": np.ascontiguousarray(xcol[E * c:E * (c + 1)]),
            "W1cb": w1cb, "Mm": mm, "W2m": w2m, "b2v": b2v,
        })
    return in_maps


def kernel(x, W1, b1, Q, K, V, W2, b2):
    from concourse.bass_utils import run_bass_kernel_spmd

    in_maps = _host_prep(x, W1, b1, Q, K, V, W2, b2)
    if "nc" not in _built:
        _built["nc"] = _build_nc()
    nc = _built["nc"]
    res = run_bass_kernel_spmd(nc, in_maps, core_ids=list(range(NCORES)))
    full = np.concatenate([res.results[c]["out"] for c in range(NCORES)], axis=0)
    return np.ascontiguousarray(
        full.reshape(x.shape[0], x.shape[1], IMG, IMG).astype(np.float32)
    )
